# revision 10
# baseline (speedup 1.0000x reference)
"""Multi-head attention (B=2, N=2048, C=1024, H=16, D=64) on 8 TRN2 NeuronCores.

Sharding: data-parallel over the 2 batches x tensor-parallel over 4 head-groups
(4 heads each) -> 8 cores, no cross-core communication. Each core computes its
QKV projection slice and full attention for its 4 heads.

Per-core kernel strategy (QKV/S in bf16, PV in f32r; ~2.4e-3 rel):
  1. x is transposed host-side; xT [1024, 2048] is DMA'd into chan-partition
     layout.
  2. qT/kT per head-pair = W_pair.T @ xT  ([128, 2048]: rows 0-63 head A,
     64-127 head B).  v = xT.T @ Wv in natural [token, dim] layout with a
     ones-column appended per head (65th wv column is zero-padded and the
     bias carries 1.0 -> denominator fusion).
  3. S^T tile [m,n] = kT_m.T @ qT_n (K=64, row-packed pair -> full array).
     exp(S/8) on ACT straight out of PSUM for both heads in one [128,1024]
     op (no max subtraction needed: logits are ~N(0, 0.4)).
     PV: out^T[d+1, n] += v_m.T @ E_m accumulated over m; row 64 is the
     softmax denominator.
  4. PE-transpose out^T chunks, multiply by reciprocal denominator, DMA out.
"""

import os

import numpy as np

import concourse.bass as bass
import concourse.tile as tile
from concourse import bacc, mybir
from concourse.bass_utils import run_bass_kernel_spmd
from concourse.masks import make_identity

f32 = mybir.dt.float32
f32r = mybir.dt.float32r
bf16 = mybir.dt.bfloat16
AF = mybir.ActivationFunctionType

B, N_TOK, C = 2, 2048, 1024
H, HD = 16, 64
SCALE = HD ** -0.5
NH = 4             # heads per core
NP = 2             # head pairs per core
GC = H // NH       # head groups (cores per batch)
CC = C // 128      # channel tiles (8)
TT = N_TOK // 128  # token tiles (16)
NB = N_TOK // 512  # n-blocks (4)
MT = N_TOK // 128  # m-tiles (16)
W_COLS = NH * HD          # 256
W_COLS_V = NH * (HD + 1)  # 260: v padded with a ones column per head


def _build(repeats=1):
    nc = bacc.Bacc("TRN2", target_bir_lowering=False, debug=False,
                   enable_asserts=False, num_devices=8)

    xT_d = nc.dram_tensor("xt", [C, N_TOK], bf16, kind="ExternalInput")
    wq_d = nc.dram_tensor("wq", [128, CC, W_COLS], bf16, kind="ExternalInput")
    wk_d = nc.dram_tensor("wk", [128, CC, W_COLS], bf16, kind="ExternalInput")
    wv_d = nc.dram_tensor("wv", [128, CC, W_COLS_V], bf16,
                          kind="ExternalInput")
    bq_d = nc.dram_tensor("bq", [128, NP], f32, kind="ExternalInput")
    bk_d = nc.dram_tensor("bk", [128, NP], f32, kind="ExternalInput")
    bv_d = nc.dram_tensor("bv", [128, W_COLS_V], f32, kind="ExternalInput")
    out_d = nc.dram_tensor("out", [N_TOK, W_COLS], f32, kind="ExternalOutput")

    with tile.TileContext(nc) as tc:
        with (
            tc.tile_pool(name="consts", bufs=1) as consts,
            tc.tile_pool(name="weights", bufs=2) as wpool,
            tc.tile_pool(name="qk", bufs=2) as qkpool,
            tc.tile_pool(name="vpool", bufs=2) as vpool,
            tc.tile_pool(name="xTp", bufs=2) as xTpool,
        ):
            ident = consts.tile([128, 128], f32, tag="ident")
            make_identity(nc, ident[:])
            bq_s = consts.tile([128, NP], f32, tag="bq")
            bk_s = consts.tile([128, NP], f32, tag="bk")
            bv_s = consts.tile([128, W_COLS_V], f32, tag="bv")
            it_n = [0]

            def _phases():
                # per-iteration tile allocation from 2-buf pools: the next
                # iteration's input DMA + projection prefix can overlap this
                # iteration's attention phase instead of waiting for the last
                # readers of the single buffers
                it = it_n[0]
                it_n[0] += 1
                qTp = [qkpool.tile([128, N_TOK], bf16, tag=f"qT{p}",
                                   name=f"qT{p}_{it}") for p in range(NP)]
                kTp = [qkpool.tile([128, N_TOK], bf16, tag=f"kT{p}",
                                   name=f"kT{p}_{it}") for p in range(NP)]
                vSt = [vpool.tile([128, W_COLS_V], f32r, tag=f"vS{tt}",
                                  name=f"vS{tt}_{it}") for tt in range(TT)]
                xTc = [xTpool.tile([128, N_TOK], bf16, tag=f"xT{cc}",
                                   name=f"xT{cc}_{it}") for cc in range(CC)]
                # -------- input DMA: weights first (small, needed first),
                # xT split across both physical HWDGE rings (SP + ACT) --------
                nc.sync.dma_start(out=bq_s[:], in_=bq_d.ap())
                nc.sync.dma_start(out=bk_s[:], in_=bk_d.ap())
                nc.sync.dma_start(out=bv_s[:], in_=bv_d.ap())

                wq_s = wpool.tile([128, CC, W_COLS], bf16, tag="wq")
                wk_s = wpool.tile([128, CC, W_COLS], bf16, tag="wk")
                wv_s = wpool.tile([128, CC, W_COLS_V], bf16, tag="wv")
                nc.sync.dma_start(out=wq_s[:], in_=wq_d.ap())
                nc.scalar.dma_start(out=wk_s[:], in_=wk_d.ap())
                nc.scalar.dma_start(out=wv_s[:], in_=wv_d.ap())
                for cc in range(CC):
                    eng = nc.sync if cc % 2 == 0 else nc.scalar
                    eng.dma_start(
                        out=xTc[cc][:],
                        in_=xT_d.ap()[cc * 128:(cc + 1) * 128, :],
                    )
                warm = consts.tile([128, 1], f32, tag="warm")
                nc.scalar.activation(warm[:], bq_s[:, 0:1], AF.Exp, scale=SCALE)
                with (
                    tc.tile_pool(name="psum", bufs=2, space="PSUM") as psum,
                    tc.tile_pool(name="epool", bufs=8) as epool,
                    tc.tile_pool(name="opool", bufs=3) as opool,
                ):
                    def group_steps(w_s, dst, b_s, pair, tth, tag="pqk"):
                        # one q-or-k projection group: 2 psums (one per token
                        # block) accumulated over cc with the W tile held
                        # stationary for 2 consecutive matmuls; yields after
                        # each cc so it can be woven into attention hooks
                        psums = [
                            psum.tile([128, 512], f32, tag=tag,
                                      name=f"g{pair}_{dst[0].name}_{tth}_{t}")
                            for t in range(2)
                        ]
                        for cc in range(CC):
                            for t in range(2):
                                ttb = tth * 2 + t
                                nc.tensor.matmul(
                                    psums[t][:],
                                    w_s[:, cc, pair * 128:(pair + 1) * 128],
                                    xTc[cc][:, ttb * 512:(ttb + 1) * 512],
                                    start=(cc == 0), stop=(cc == CC - 1),
                                )
                            yield
                        with nc.allow_low_precision(reason="bf16 qk"):
                            for t in range(2):
                                ttb = tth * 2 + t
                                nc.vector.tensor_scalar_add(
                                    dst[pair][:, ttb * 512:(ttb + 1) * 512],
                                    psums[t][:], b_s[:, pair:pair + 1],
                                )
                        yield

                    def v_all(tt_lo, tt_hi):
                        for tt in range(tt_lo, tt_hi):
                            pv = psum.tile([128, W_COLS_V], f32, tag="pqk",
                                           name=f"pv_{tt}")
                            for cc in range(CC):
                                nc.tensor.matmul(
                                    pv[:],
                                    xTc[cc][:, tt * 128:(tt + 1) * 128],
                                    wv_s[:, cc, :],
                                    start=(cc == 0), stop=(cc == CC - 1),
                                )
                            nc.vector.tensor_add(vSt[tt][:], pv[:], bv_s[:])

                    def attn_nb(pair, nb, hook=None, pending_tail=None):
                        hA, hB = 2 * pair, 2 * pair + 1
                        po_A = psum.tile([65, 512], f32, tag="po",
                                         name=f"po_A_{pair}_{nb}")
                        po_B = psum.tile([65, 512], f32, tag="po",
                                         name=f"po_B_{pair}_{nb}")
                        nq = nb * 512
                        for mt in range(MT):
                            if mt == 2 and pending_tail is not None:
                                # previous block's deferred output tail: runs
                                # here so the new block's first S matmuls (and
                                # their exps) precede it in the PE stream,
                                # keeping ACT fed across the block boundary
                                pending_tail()
                                pending_tail = None
                            ps = psum.tile([128, 1024], f32, tag="ps",
                                           name=f"ps_{pair}_{nb}_{mt}")
                            nc.tensor.matmul(
                                ps[:, 0:512],
                                kTp[pair][0:64, mt * 128:(mt + 1) * 128],
                                qTp[pair][0:64, nq:nq + 512],
                                start=True, stop=True,
                            )
                            nc.tensor.matmul(
                                ps[:, 512:1024],
                                kTp[pair][64:128, mt * 128:(mt + 1) * 128],
                                qTp[pair][64:128, nq:nq + 512],
                                start=True, stop=True,
                            )
                            ee = epool.tile([128, 1024], f32r, tag="ee")
                            nc.scalar.activation(ee[:], ps[:], AF.Exp, scale=SCALE)
                            nc.tensor.matmul(
                                po_A[:], vSt[mt][:, hA * 65:(hA + 1) * 65],
                                ee[:, 0:512],
                                start=(mt == 0), stop=(mt == MT - 1),
                            )
                            nc.tensor.matmul(
                                po_B[:], vSt[mt][:, hB * 65:(hB + 1) * 65],
                                ee[:, 512:1024],
                                start=(mt == 0), stop=(mt == MT - 1),
                            )
                            if hook is not None:
                                hook(mt)
                        osbs = []
                        for head, po in ((hA, po_A), (hB, po_B)):
                            osb = opool.tile([65, 512], f32, tag="osb")
                            nc.vector.tensor_copy(osb[:], po[:])
                            osbs.append((head, osb))

                        def tail():
                            for head, osb in osbs:
                                # pot borrows a "ps" slot: it is freed by ACT
                                # (exp), which depends only on earlier PE
                                # work, so deferring this into the next
                                # block's stream cannot deadlock
                                pot = psum.tile([128, 4, 65], f32, tag="ps",
                                                name=f"pot_{pair}_{nb}_{head}")
                                for j in range(4):
                                    nc.tensor.transpose(
                                        pot[:, j, :],
                                        osb[:, j * 128:(j + 1) * 128],
                                        ident[0:65, 0:65],
                                    )
                                rc = opool.tile([128, 4], f32, tag="rc")
                                nc.vector.reciprocal(rc[:], pot[:, :, 64])
                                fo = opool.tile([128, 4, HD], f32, tag="fo")
                                for j in range(4):
                                    nc.vector.tensor_scalar_mul(
                                        fo[:, j, :], pot[:, j, 0:HD],
                                        rc[:, j:j + 1]
                                    )
                                nc.sync.dma_start(
                                    out=out_d.ap()[nq:nq + 512,
                                                   head * HD:(head + 1) * HD]
                                    .rearrange("(j p) d -> p j d", p=128),
                                    in_=fo[:],
                                )
                        return tail


                    def run_all(gen):
                        for _ in gen:
                            pass

                    def fill_gen():
                        # PE work woven into attention hooks, in dependency
                        # order: v[mt] is emitted 2 hook-slots before PV(nb0,
                        # mt) needs it; qT0 blocks 2-3 complete during nb1
                        # (read by nb2+); all of qk(pair1) completes during
                        # attn(pair0).
                        for tt in range(2, TT):
                            v_all(tt, tt + 1)
                            yield
                        yield from group_steps(wq_s, qTp, bq_s, 0, 1)
                        for tth in range(2):
                            yield from group_steps(wk_s, kTp, bk_s, 1, tth)
                        for tth in range(2):
                            yield from group_steps(wq_s, qTp, bq_s, 1, tth)

                    # minimal critical prefix before attention: kT(pair0)
                    # in full, qT(pair0) blocks 0-1, v[0..1].  The three
                    # groups advance chunk-by-chunk in lockstep (each on its
                    # own borrowed PSUM tag) so every arriving xT chunk
                    # unblocks all of them and the PE never idles long enough
                    # to fall out of its warm clock state.
                    g1 = group_steps(wk_s, kTp, bk_s, 0, 0, tag="pqk")
                    g2 = group_steps(wk_s, kTp, bk_s, 0, 1, tag="ps")
                    g3 = group_steps(wq_s, qTp, bq_s, 0, 0, tag="po")
                    for cc in range(CC):
                        next(g1)
                        next(g2)
                        next(g3)
                    run_all(g1)
                    run_all(g2)
                    run_all(g3)
                    v_all(0, 2)
                    gen = fill_gen()
                    hook = lambda mt: next(gen, None)
                    tail = None
                    for pair in range(NP):
                        for nb in range(NB):
                            tail = attn_nb(pair, nb, hook=hook,
                                           pending_tail=tail)
                    tail()
            if repeats == 1:
                _phases()
            elif repeats % 2 == 0:
                # unroll by 2: the two copies allocate alternating buffer
                # sets from the 2-buf pools, so each copy's input DMA +
                # projection prefix overlaps the other copy's attention phase
                with tc.For_i(0, repeats // 2, 1):
                    _phases()
                    _phases()
            else:
                with tc.For_i(0, repeats, 1):
                    _phases()

    nc.compile()
    return nc


_NC = None


def _get_nc():
    global _NC
    if _NC is None:
        _NC = _build(repeats=int(os.environ.get("KERNEL_REPEATS", "1")))
    return _NC


def _in_maps(x, w_qkv, b_qkv):
    import ml_dtypes
    x = np.ascontiguousarray(x, dtype=np.float32)
    w_qkv = np.ascontiguousarray(w_qkv, dtype=np.float32)
    b_qkv = np.ascontiguousarray(b_qkv, dtype=np.float32)
    xts = [np.ascontiguousarray(x[b].T).astype(ml_dtypes.bfloat16)
           for b in range(B)]
    maps = []
    for core in range(8):
        b = core // GC
        g = core % GC
        cols = slice(g * W_COLS, (g + 1) * W_COLS)
        wq = w_qkv[:, 0 * C:1 * C][:, cols]
        wk = w_qkv[:, 1 * C:2 * C][:, cols]
        wv_raw = w_qkv[:, 2 * C:3 * C][:, cols]
        wv = np.zeros((C, W_COLS_V), dtype=np.float32)
        wv.reshape(C, NH, HD + 1)[:, :, 0:HD] = wv_raw.reshape(C, NH, HD)
        # [c, m] -> [p, cc, m] so the on-device DMA is fully contiguous
        wq = wq.reshape(CC, 128, W_COLS).transpose(1, 0, 2)
        wk = wk.reshape(CC, 128, W_COLS).transpose(1, 0, 2)
        wv = wv.reshape(CC, 128, W_COLS_V).transpose(1, 0, 2)
        bq = b_qkv[0 * C:1 * C][cols].reshape(NP, 128).T
        bk = b_qkv[1 * C:2 * C][cols].reshape(NP, 128).T
        bv_row = np.zeros((W_COLS_V,), dtype=np.float32)
        bv_row.reshape(NH, HD + 1)[:, 0:HD] = b_qkv[2 * C:3 * C][cols].reshape(NH, HD)
        bv_row.reshape(NH, HD + 1)[:, HD] = 1.0
        bv = np.broadcast_to(bv_row, (128, W_COLS_V))
        import ml_dtypes as _md
        maps.append({
            "xt": xts[b],
            "wq": np.ascontiguousarray(wq).astype(_md.bfloat16),
            "wk": np.ascontiguousarray(wk).astype(_md.bfloat16),
            "wv": wv.astype(_md.bfloat16),
            "bq": np.ascontiguousarray(bq),
            "bk": np.ascontiguousarray(bk),
            "bv": np.ascontiguousarray(bv),
        })
    return maps


def kernel(x, w_qkv, b_qkv):
    nc = _get_nc()
    maps = _in_maps(x, w_qkv, b_qkv)
    res = run_bass_kernel_spmd(nc, maps, list(range(8)))
    y = np.empty((B, N_TOK, C), dtype=np.float32)
    for core in range(8):
        b = core // GC
        g = core % GC
        y[b, :, g * W_COLS:(g + 1) * W_COLS] = res.results[core]["out"]
    return y



# revision 11
# speedup vs baseline: 1.1622x; 1.1622x over previous
"""Multi-head attention (B=2, N=2048, C=1024, H=16, D=64) on 8 TRN2 NeuronCores.

Sharding: data-parallel over the 2 batches x tensor-parallel over 4 head-groups
(4 heads each) -> 8 cores, no cross-core communication. Each core computes its
QKV projection slice and full attention for its 4 heads.

Per-core kernel strategy (QKV/S in bf16, PV in f32r; ~2.4e-3 rel):
  1. x is transposed host-side; xT [1024, 2048] is DMA'd into chan-partition
     layout.
  2. qT/kT per head-pair = W_pair.T @ xT  ([128, 2048]: rows 0-63 head A,
     64-127 head B).  v = xT.T @ Wv in natural [token, dim] layout with a
     ones-column appended per head (65th wv column is zero-padded and the
     bias carries 1.0 -> denominator fusion).
  3. S^T tile [m,n] = kT_m.T @ qT_n (K=64, row-packed pair -> full array).
     exp(S/8) on ACT straight out of PSUM for both heads in one [128,1024]
     op (no max subtraction needed: logits are ~N(0, 0.4)).
     PV: out^T[d+1, n] += v_m.T @ E_m accumulated over m; row 64 is the
     softmax denominator.
  4. PE-transpose out^T chunks, multiply by reciprocal denominator, DMA out.
"""

import os

import numpy as np

import concourse.bass as bass
import concourse.tile as tile
from concourse import bacc, mybir
from concourse.bass_utils import run_bass_kernel_spmd
from concourse.masks import make_identity

f32 = mybir.dt.float32
f32r = mybir.dt.float32r
bf16 = mybir.dt.bfloat16
AF = mybir.ActivationFunctionType

B, N_TOK, C = 2, 2048, 1024
H, HD = 16, 64
SCALE = HD ** -0.5
NH = 4             # heads per core
NP = 2             # head pairs per core
GC = H // NH       # head groups (cores per batch)
CC = C // 128      # channel tiles (8)
TT = N_TOK // 128  # token tiles (16)
NB = N_TOK // 512  # n-blocks (4)
MT = N_TOK // 128  # m-tiles (16)
W_COLS = NH * HD          # 256
W_COLS_V = NH * (HD + 1)  # 260: v padded with a ones column per head


def _build(repeats=1):
    nc = bacc.Bacc("TRN2", target_bir_lowering=False, debug=False,
                   enable_asserts=False, num_devices=8)

    xT_d = nc.dram_tensor("xt", [C, N_TOK], bf16, kind="ExternalInput")
    wq_d = nc.dram_tensor("wq", [128, CC, W_COLS], bf16, kind="ExternalInput")
    wk_d = nc.dram_tensor("wk", [128, CC, W_COLS], bf16, kind="ExternalInput")
    wv_d = nc.dram_tensor("wv", [128, CC, W_COLS_V], bf16,
                          kind="ExternalInput")
    bq_d = nc.dram_tensor("bq", [128, NP], f32, kind="ExternalInput")
    bk_d = nc.dram_tensor("bk", [128, NP], f32, kind="ExternalInput")
    bv_d = nc.dram_tensor("bv", [128, W_COLS_V], f32, kind="ExternalInput")
    out_d = nc.dram_tensor("out", [N_TOK, W_COLS], f32, kind="ExternalOutput")

    with tile.TileContext(nc) as tc:
        with (
            tc.tile_pool(name="consts", bufs=1) as consts,
            tc.tile_pool(name="weights", bufs=2) as wpool,
            tc.tile_pool(name="qk", bufs=2) as qkpool,
            tc.tile_pool(name="vpool", bufs=2) as vpool,
            tc.tile_pool(name="xTp", bufs=2) as xTpool,
        ):
            ident = consts.tile([128, 128], f32, tag="ident")
            make_identity(nc, ident[:])
            bq_s = consts.tile([128, NP], f32, tag="bq")
            bk_s = consts.tile([128, NP], f32, tag="bk")
            bv_s = consts.tile([128, W_COLS_V], f32, tag="bv")
            it_n = [0]

            def _phases():
                # per-iteration tile allocation from 2-buf pools: the next
                # iteration's input DMA + projection prefix can overlap this
                # iteration's attention phase instead of waiting for the last
                # readers of the single buffers
                it = it_n[0]
                it_n[0] += 1
                qTp = [qkpool.tile([128, N_TOK], bf16, tag=f"qT{p}",
                                   name=f"qT{p}_{it}") for p in range(NP)]
                kTp = [qkpool.tile([128, N_TOK], bf16, tag=f"kT{p}",
                                   name=f"kT{p}_{it}") for p in range(NP)]
                vSt = [vpool.tile([128, W_COLS_V], f32r, tag=f"vS{tt}",
                                  name=f"vS{tt}_{it}") for tt in range(TT)]
                xTc = [xTpool.tile([128, N_TOK], bf16, tag=f"xT{cc}",
                                   name=f"xT{cc}_{it}") for cc in range(CC)]
                # -------- input DMA: weights first (small, needed first),
                # xT split across both physical HWDGE rings (SP + ACT) --------
                nc.sync.dma_start(out=bq_s[:], in_=bq_d.ap())
                nc.sync.dma_start(out=bk_s[:], in_=bk_d.ap())
                nc.sync.dma_start(out=bv_s[:], in_=bv_d.ap())

                wq_s = wpool.tile([128, CC, W_COLS], bf16, tag="wq")
                wk_s = wpool.tile([128, CC, W_COLS], bf16, tag="wk")
                wv_s = wpool.tile([128, CC, W_COLS_V], bf16, tag="wv")
                nc.sync.dma_start(out=wq_s[:], in_=wq_d.ap())
                nc.scalar.dma_start(out=wk_s[:], in_=wk_d.ap())
                nc.scalar.dma_start(out=wv_s[:], in_=wv_d.ap())
                for cc in range(CC):
                    eng = nc.sync if cc % 2 == 0 else nc.scalar
                    eng.dma_start(
                        out=xTc[cc][:],
                        in_=xT_d.ap()[cc * 128:(cc + 1) * 128, :],
                    )
                warm = consts.tile([128, 1], f32, tag="warm")
                nc.scalar.activation(warm[:], bq_s[:, 0:1], AF.Exp, scale=SCALE)
                with (
                    tc.tile_pool(name="psum", bufs=2, space="PSUM") as psum,
                    tc.tile_pool(name="epool", bufs=6) as epool,
                    tc.tile_pool(name="opool", bufs=2) as opool,
                ):
                    def group_steps(w_s, dst, b_s, pair, tth, tag="pqk"):
                        # one q-or-k projection group: 2 psums (one per token
                        # block) accumulated over cc with the W tile held
                        # stationary for 2 consecutive matmuls; yields after
                        # each cc so it can be woven into attention hooks
                        psums = [
                            psum.tile([128, 512], f32, tag=tag,
                                      name=f"g{pair}_{dst[0].name}_{tth}_{t}")
                            for t in range(2)
                        ]
                        for cc in range(CC):
                            for t in range(2):
                                ttb = tth * 2 + t
                                nc.tensor.matmul(
                                    psums[t][:],
                                    w_s[:, cc, pair * 128:(pair + 1) * 128],
                                    xTc[cc][:, ttb * 512:(ttb + 1) * 512],
                                    start=(cc == 0), stop=(cc == CC - 1),
                                )
                            yield
                        with nc.allow_low_precision(reason="bf16 qk"):
                            for t in range(2):
                                ttb = tth * 2 + t
                                nc.vector.tensor_scalar_add(
                                    dst[pair][:, ttb * 512:(ttb + 1) * 512],
                                    psums[t][:], b_s[:, pair:pair + 1],
                                )
                        yield

                    def v_all(tt_lo, tt_hi):
                        for tt in range(tt_lo, tt_hi):
                            pv = psum.tile([128, W_COLS_V], f32, tag="pqk",
                                           name=f"pv_{tt}")
                            for cc in range(CC):
                                nc.tensor.matmul(
                                    pv[:],
                                    xTc[cc][:, tt * 128:(tt + 1) * 128],
                                    wv_s[:, cc, :],
                                    start=(cc == 0), stop=(cc == CC - 1),
                                )
                            nc.vector.tensor_add(vSt[tt][:], pv[:], bv_s[:])

                    def attn_nb(pair, nb, hook=None, pending_tail=None):
                        hA, hB = 2 * pair, 2 * pair + 1
                        po_A = psum.tile([65, 512], f32, tag="po",
                                         name=f"po_A_{pair}_{nb}")
                        po_B = psum.tile([65, 512], f32, tag="po",
                                         name=f"po_B_{pair}_{nb}")
                        nq = nb * 512
                        for mt in range(MT):
                            if mt == 2 and pending_tail is not None:
                                # previous block's deferred output tail: runs
                                # here so the new block's first S matmuls (and
                                # their exps) precede it in the PE stream,
                                # keeping ACT fed across the block boundary
                                pending_tail()
                                pending_tail = None
                            ps = psum.tile([128, 1024], f32, tag="ps",
                                           name=f"ps_{pair}_{nb}_{mt}")
                            nc.tensor.matmul(
                                ps[:, 0:512],
                                kTp[pair][0:64, mt * 128:(mt + 1) * 128],
                                qTp[pair][0:64, nq:nq + 512],
                                start=True, stop=True,
                            )
                            nc.tensor.matmul(
                                ps[:, 512:1024],
                                kTp[pair][64:128, mt * 128:(mt + 1) * 128],
                                qTp[pair][64:128, nq:nq + 512],
                                start=True, stop=True,
                            )
                            ee = epool.tile([128, 1024], f32r, tag="ee")
                            nc.scalar.activation(ee[:], ps[:], AF.Exp, scale=SCALE)
                            nc.tensor.matmul(
                                po_A[:], vSt[mt][:, hA * 65:(hA + 1) * 65],
                                ee[:, 0:512],
                                start=(mt == 0), stop=(mt == MT - 1),
                            )
                            nc.tensor.matmul(
                                po_B[:], vSt[mt][:, hB * 65:(hB + 1) * 65],
                                ee[:, 512:1024],
                                start=(mt == 0), stop=(mt == MT - 1),
                            )
                            if hook is not None:
                                hook(mt)
                        osbs = []
                        for head, po in ((hA, po_A), (hB, po_B)):
                            osb = opool.tile([65, 512], f32, tag="osb")
                            nc.vector.tensor_copy(osb[:], po[:])
                            osbs.append((head, osb))

                        def tail():
                            for head, osb in osbs:
                                # pot borrows a "ps" slot: it is freed by ACT
                                # (exp), which depends only on earlier PE
                                # work, so deferring this into the next
                                # block's stream cannot deadlock
                                pot = psum.tile([128, 4, 65], f32, tag="ps",
                                                name=f"pot_{pair}_{nb}_{head}")
                                for j in range(4):
                                    nc.tensor.transpose(
                                        pot[:, j, :],
                                        osb[:, j * 128:(j + 1) * 128],
                                        ident[0:65, 0:65],
                                    )
                                rc = opool.tile([128, 4], f32, tag="rc")
                                nc.vector.reciprocal(rc[:], pot[:, :, 64])
                                fo = opool.tile([128, 4, HD], f32, tag="fo")
                                for j in range(4):
                                    nc.vector.tensor_scalar_mul(
                                        fo[:, j, :], pot[:, j, 0:HD],
                                        rc[:, j:j + 1]
                                    )
                                nc.sync.dma_start(
                                    out=out_d.ap()[nq:nq + 512,
                                                   head * HD:(head + 1) * HD]
                                    .rearrange("(j p) d -> p j d", p=128),
                                    in_=fo[:],
                                )
                        return tail


                    def run_all(gen):
                        for _ in gen:
                            pass

                    def fill_gen():
                        # PE work woven into attention hooks, in dependency
                        # order: v[mt] is emitted 2 hook-slots before PV(nb0,
                        # mt) needs it; qT0 blocks 2-3 complete during nb1
                        # (read by nb2+); all of qk(pair1) completes during
                        # attn(pair0).
                        for tt in range(2, TT):
                            v_all(tt, tt + 1)
                            yield
                        yield from group_steps(wq_s, qTp, bq_s, 0, 1)
                        for tth in range(2):
                            yield from group_steps(wk_s, kTp, bk_s, 1, tth)
                        for tth in range(2):
                            yield from group_steps(wq_s, qTp, bq_s, 1, tth)

                    # minimal critical prefix before attention: kT(pair0)
                    # in full, qT(pair0) blocks 0-1, v[0..1].  The three
                    # groups advance chunk-by-chunk in lockstep (each on its
                    # own borrowed PSUM tag) so every arriving xT chunk
                    # unblocks all of them and the PE never idles long enough
                    # to fall out of its warm clock state.
                    g1 = group_steps(wk_s, kTp, bk_s, 0, 0, tag="pqk")
                    g2 = group_steps(wk_s, kTp, bk_s, 0, 1, tag="ps")
                    g3 = group_steps(wq_s, qTp, bq_s, 0, 0, tag="po")
                    for cc in range(CC):
                        next(g1)
                        next(g2)
                        next(g3)
                    run_all(g1)
                    run_all(g2)
                    run_all(g3)
                    v_all(0, 2)
                    gen = fill_gen()
                    hook = lambda mt: next(gen, None)
                    tail = None
                    for pair in range(NP):
                        for nb in range(NB):
                            tail = attn_nb(pair, nb, hook=hook,
                                           pending_tail=tail)
                    tail()
            if repeats == 1:
                _phases()
            elif repeats % 2 == 0:
                # unroll by 2: the two copies allocate alternating buffer
                # sets from the 2-buf pools, so each copy's input DMA +
                # projection prefix overlaps the other copy's attention phase
                with tc.For_i(0, repeats // 2, 1):
                    _phases()
                    _phases()
            else:
                with tc.For_i(0, repeats, 1):
                    _phases()

    nc.compile()
    return nc


_NC = None


def _get_nc():
    global _NC
    if _NC is None:
        _NC = _build(repeats=int(os.environ.get("KERNEL_REPEATS", "1")))
    return _NC


def _in_maps(x, w_qkv, b_qkv):
    import ml_dtypes
    x = np.ascontiguousarray(x, dtype=np.float32)
    w_qkv = np.ascontiguousarray(w_qkv, dtype=np.float32)
    b_qkv = np.ascontiguousarray(b_qkv, dtype=np.float32)
    xts = [np.ascontiguousarray(x[b].T).astype(ml_dtypes.bfloat16)
           for b in range(B)]
    maps = []
    for core in range(8):
        b = core // GC
        g = core % GC
        cols = slice(g * W_COLS, (g + 1) * W_COLS)
        wq = w_qkv[:, 0 * C:1 * C][:, cols]
        wk = w_qkv[:, 1 * C:2 * C][:, cols]
        wv_raw = w_qkv[:, 2 * C:3 * C][:, cols]
        wv = np.zeros((C, W_COLS_V), dtype=np.float32)
        wv.reshape(C, NH, HD + 1)[:, :, 0:HD] = wv_raw.reshape(C, NH, HD)
        # [c, m] -> [p, cc, m] so the on-device DMA is fully contiguous
        wq = wq.reshape(CC, 128, W_COLS).transpose(1, 0, 2)
        wk = wk.reshape(CC, 128, W_COLS).transpose(1, 0, 2)
        wv = wv.reshape(CC, 128, W_COLS_V).transpose(1, 0, 2)
        bq = b_qkv[0 * C:1 * C][cols].reshape(NP, 128).T
        bk = b_qkv[1 * C:2 * C][cols].reshape(NP, 128).T
        bv_row = np.zeros((W_COLS_V,), dtype=np.float32)
        bv_row.reshape(NH, HD + 1)[:, 0:HD] = b_qkv[2 * C:3 * C][cols].reshape(NH, HD)
        bv_row.reshape(NH, HD + 1)[:, HD] = 1.0
        bv = np.broadcast_to(bv_row, (128, W_COLS_V))
        import ml_dtypes as _md
        maps.append({
            "xt": xts[b],
            "wq": np.ascontiguousarray(wq).astype(_md.bfloat16),
            "wk": np.ascontiguousarray(wk).astype(_md.bfloat16),
            "wv": wv.astype(_md.bfloat16),
            "bq": np.ascontiguousarray(bq),
            "bk": np.ascontiguousarray(bk),
            "bv": np.ascontiguousarray(bv),
        })
    return maps


def kernel(x, w_qkv, b_qkv):
    nc = _get_nc()
    maps = _in_maps(x, w_qkv, b_qkv)
    res = run_bass_kernel_spmd(nc, maps, list(range(8)))
    y = np.empty((B, N_TOK, C), dtype=np.float32)
    for core in range(8):
        b = core // GC
        g = core % GC
        y[b, :, g * W_COLS:(g + 1) * W_COLS] = res.results[core]["out"]
    return y

